# revision 12
# baseline (speedup 1.0000x reference)
"""Trainium2 Bass kernel for nn_MixedFeedForward (shared MLP + 16 per-ns-token MLPs).

Sharding (8 NeuronCores, SPMD, no collectives):
  - shared path: data-parallel over batch -> core i runs the shared MLP over
    x[i, :1024, :].
  - ns path: expert-parallel -> core i runs experts {2i, 2i+1}, each over the
    8 batches' single ns token for that expert.
Each core writes a disjoint slice of the output; the host assembles.

Shared path is bf16 (PE streaming floor ~221us/core); expert path is
fp8e4 + DoubleRow (halves the 1024 expert matmuls; ns tokens are 16/1040
of the output so the fp8 noise contributes ~7e-3 global rel err).

Schedule (all paced by emission order, PE FIFO):
  - ~8 dummy N=512 matmuls at t~7us warm the HAM clock gate before real
    work; short dummy fillers in the first f-blocks keep it warm through
    the DMA-starved start.
  - L1 runs token-block 0 across all 8 f-blocks, then token-block 1
    (W1_seq held resident in SBUF) so the early DMA demand is just
    256KB/1.7us after the first group.
  - Expert L1/L2 DoubleRow matmuls are Bresenham-paced between shared
    matmuls (~1 per 1.75) so their 256-col LDWEIGHTS hide under the
    216ns shared streams; expert output is written during chunk 7.
"""

import os
import sys
import numpy as np
import ml_dtypes

BF16 = ml_dtypes.bfloat16
E4 = ml_dtypes.float8_e4m3

P = 128
D_MODEL, D_FF = 1024, 4096
SEQ_TOK, NS_TOK, BATCH = 1024, 16, 8
SEQ_LEN = SEQ_TOK + NS_TOK
N_CORES = 8
E_PER_CORE = 2
KO1 = D_MODEL // P      # 8  k-chunks when contracting over d_model
KO2 = D_FF // P         # 32 k-chunks when contracting over d_ff
FBLK = D_FF // 512      # 8  f-blocks (512 wide)
TBLK = SEQ_TOK // 512   # 2  token blocks (512 wide)
XS, W1S, W2S = 16.0, 32.0, 64.0   # fp8 pre-scales (powers of 2)

_state = {}


def _ensure_axon_profile_hook():
    """Some agent images lack antenv.axon_hooks; provide a shim so
    run_bass_kernel_spmd(trace=True) can capture NTFF profiles via the
    libaxon_pjrt C ABI (same mechanism as trn_agent_boot)."""
    try:
        import antenv.axon_hooks  # noqa: F401
        return
    except ImportError:
        pass
    import contextlib
    import ctypes
    import types

    so_path = "/opt/axon/libaxon_pjrt.so"
    hook = None
    if os.path.exists(so_path):
        try:
            lib = ctypes.CDLL(so_path)
            if hasattr(lib, "axon_start_nrt_profile"):
                lib.axon_start_nrt_profile.argtypes = [
                    ctypes.POINTER(ctypes.c_int64), ctypes.c_size_t]
                lib.axon_start_nrt_profile.restype = ctypes.c_int64
                lib.axon_stop_nrt_profile.argtypes = [ctypes.c_char_p]
                lib.axon_stop_nrt_profile.restype = ctypes.c_int64

                @contextlib.contextmanager
                def _hook(output_dir, device_ids):
                    import jax
                    jax.devices()
                    if device_ids:
                        ids = (ctypes.c_int64 * len(device_ids))(*device_ids)
                        rc = lib.axon_start_nrt_profile(ids, len(device_ids))
                    else:
                        rc = lib.axon_start_nrt_profile(None, 0)
                    if rc != 0:
                        raise RuntimeError(f"axon_start_nrt_profile rc={rc}")
                    try:
                        yield
                    finally:
                        n = lib.axon_stop_nrt_profile(str(output_dir).encode())
                        print(f"profile: {n} file(s) written to {output_dir}",
                              file=sys.stderr)

                hook = _hook
        except OSError:
            pass

    mod = types.ModuleType("antenv.axon_hooks")
    _store = {"hook": hook}
    mod.set_axon_ntff_profile_hook = lambda h: _store.__setitem__("hook", h)
    mod.get_axon_ntff_profile_hook = lambda: _store["hook"]
    sys.modules["antenv.axon_hooks"] = mod


_ensure_axon_profile_hook()


def _build():
    import concourse.mybir as mybir
    import concourse.tile as tile
    from concourse import bacc

    f32 = mybir.dt.float32
    bf16 = mybir.dt.bfloat16
    fp8 = mybir.dt.float8e4
    AF = mybir.ActivationFunctionType
    DR = mybir.MatmulPerfMode.DoubleRow

    nc = bacc.Bacc(None, target_bir_lowering=False, debug=False)

    # piece-major DRAM layouts: every weight/x DMA below is one fully
    # contiguous read
    xbp = nc.dram_tensor("xbp", [TBLK, P, KO1, 512], bf16, kind="ExternalInput")
    xnsT8 = nc.dram_tensor("xnsT8", [P, KO1, E_PER_CORE * BATCH], fp8,
                           kind="ExternalInput")
    # W1 pieces are fs-major so the warm-up can stream 256KB sub-pieces
    w1sp = nc.dram_tensor("w1sp", [FBLK, P, 4, KO1, 128], bf16, kind="ExternalInput")
    w2sp = nc.dram_tensor("w2sp", [8, P, KO2, 128], bf16, kind="ExternalInput")
    w1ep8 = nc.dram_tensor("w1ep8", [E_PER_CORE, FBLK, P, 4, KO1, 128], fp8,
                           kind="ExternalInput")
    w2ep8 = nc.dram_tensor("w2ep8", [E_PER_CORE, KO1, P, KO2, 128], fp8,
                           kind="ExternalInput")
    # all per-partition bias constants packed into one contiguous DMA:
    # cols [0:32)=b1s [32:40)=b2s [40:72)=b1e0 [72:104)=b1e1
    #      [104:112)=b2e0 [112:120)=b2e1
    consts = nc.dram_tensor("consts", [P, 120], f32, kind="ExternalInput")
    outsT = nc.dram_tensor("outsT", [D_MODEL, SEQ_TOK], bf16, kind="ExternalOutput")
    outnsT = nc.dram_tensor("outnsT", [P, KO1, E_PER_CORE * BATCH], bf16,
                            kind="ExternalOutput")

    with tile.TileContext(nc) as tc:
        with tc.tile_pool(name="main", bufs=1) as pool, \
             tc.tile_pool(name="psum", bufs=1, space="PSUM") as pp:

            # ---- HAM pre-warm: dummy matmuls with no DMA deps keep the PE
            # busy from ~8us so the clock gate is at 8/8 when real MMs start.
            # psd shares the pse2 bank (idle until L2). gpsimd memset runs
            # ~0.7us earlier than vector (shorter preamble).
            dummy = pool.tile([P, 512], bf16, tag="dummy", bufs=1)
            nc.gpsimd.memset(dummy, 0.0)
            psd = pp.tile([P, 512], f32, tag="pse2", bufs=2, name="psd")
            for i in range(10):
                nc.tensor.matmul(psd, dummy[:, 0:128], dummy,
                                 start=True, stop=True)

            def dummy_mm():
                # short (N=128) filler to keep the PE busy across early
                # DMA-starved stalls without delaying real MMs much
                nc.tensor.matmul(psd[:, 0:128], dummy[:, 0:128],
                                 dummy[:, 0:128], start=True, stop=True)

            # ---- persistent activations / resident weights ---------------
            xb = pool.tile([P, TBLK, KO1, 512], bf16, tag="xb", bufs=1)
            hT = pool.tile([P, KO2, SEQ_TOK], bf16, tag="hT", bufs=1)
            heT8 = []
            for le in range(E_PER_CORE):
                t = pool.tile([P, KO2, BATCH], fp8, tag=f"heT{le}", bufs=1,
                              name=f"heT{le}")
                heT8.append(t)
            w1t = [pool.tile([P, 4, KO1, 128], bf16, tag="w1res", bufs=FBLK,
                             name=f"w1res{fb}") for fb in range(FBLK)]

            # ---- warm-up DMAs, dispatched on 3 engines in parallel -------
            # (each dma_start costs ~0.7us of serial DIRECT2D dispatch on
            # its engine; sync/scalar/gpsimd chains run concurrently)
            nc.sync.dma_start(out=xb[:, 0, 0:2, :], in_=xbp[0][:, 0:2])
            nc.scalar.dma_start(out=w1t[0][:, 0], in_=w1sp[0][:, 0])
            nc.sync.dma_start(out=xb[:, 0, 2:4, :], in_=xbp[0][:, 2:4])
            nc.scalar.dma_start(out=w1t[0][:, 1], in_=w1sp[0][:, 1])
            nc.sync.dma_start(out=xb[:, 0, 4:6, :], in_=xbp[0][:, 4:6])
            nc.scalar.dma_start(out=w1t[0][:, 2], in_=w1sp[0][:, 2])
            nc.sync.dma_start(out=xb[:, 0, 6:8, :], in_=xbp[0][:, 6:8])
            nc.scalar.dma_start(out=w1t[0][:, 3], in_=w1sp[0][:, 3])
            cs = pool.tile([P, 120], f32, tag="consts", bufs=1)
            nc.gpsimd.dma_start(out=cs, in_=consts[:])
            b1s_sb = cs[:, 0:32]
            b2s_sb = cs[:, 32:40]
            b1e_sb = [cs[:, 40:72], cs[:, 72:104]]
            b2e_sb = [cs[:, 104:112], cs[:, 112:120]]
            xnsb8 = pool.tile([P, KO1, E_PER_CORE * BATCH], fp8, tag="xnsb8",
                              bufs=1)
            nc.gpsimd.dma_start(out=xnsb8, in_=xnsT8[:])

            # ---- expert L1: fp8 DoubleRow stream (one yield per MM) ------
            l1_rounds = [(le, rfb) for le in range(E_PER_CORE)
                         for rfb in range(FBLK)]

            def load_we8(le, rfb):
                t = pool.tile([P, 4, KO1, 128], fp8, tag="wb8", bufs=3,
                              name=f"we8_{le}_{rfb}")
                nc.gpsimd.dma_start(out=t, in_=w1ep8[le, rfb])
                return t

            def expert_l1_gen():
                nxt = load_we8(*l1_rounds[0])
                yield  # priming: weights for round 0 issued, no MM yet
                for idx, (le, rfb) in enumerate(l1_rounds):
                    web = nxt
                    nxt = (load_we8(*l1_rounds[idx + 1])
                           if idx + 1 < len(l1_rounds) else None)
                    for fs in range(4):
                        fc = rfb * 4 + fs
                        pse = pp.tile([P, BATCH], f32, tag="pse1", bufs=2,
                                      name=f"pse1_{le}_{fc}")
                        for k2 in range(0, KO1, 2):
                            nc.tensor.matmul(
                                pse,
                                web[:, fs, k2:k2 + 2, :],
                                xnsb8[:, k2:k2 + 2,
                                      le * BATCH:(le + 1) * BATCH],
                                start=(k2 == 0), stop=(k2 == KO1 - 2),
                                perf_mode=DR)
                            yield
                        nc.scalar.activation(
                            heT8[le][:, fc, :], pse, AF.Gelu,
                            bias=b1e_sb[le][:, fc:fc + 1],
                            scale=1.0 / (XS * W1S))

            el1 = expert_l1_gen()
            next(el1, None)  # prime: round-0 expert weights load (gpsimd)

            # ---- layer 1: tb0 sweep then tb1 sweep (W1 resident) ---------
            # Expert DR tiles strictly every 2nd shared MM (a 256-col DR
            # LDWEIGHTS only hides under a full 216ns shared MM stream);
            # extra stride-1 slots late in pass2 absorb the remainder.
            for tb in range(TBLK):
                for fb in range(FBLK):
                    for fs in range(4):
                        fc = fb * 4 + fs
                        ps1 = pp.tile([P, 512], f32, tag="ps1", bufs=2,
                                      name=f"ps1_{fc}_{tb}")
                        for k in range(KO1):
                            nc.tensor.matmul(
                                ps1,
                                w1t[fb][:, fs, k, :],
                                xb[:, tb, k, :],
                                start=(k == 0), stop=(k == KO1 - 1))
                            if tb == 0 and fb == 0:
                                if k % 4 == 3:
                                    dummy_mm()
                            elif k % 2 == 1 or (tb == 1 and fb >= 6):
                                next(el1, None)
                        nc.scalar.activation(
                            hT[:, fc, tb * 512:(tb + 1) * 512], ps1, AF.Gelu,
                            bias=b1s_sb[:, fc:fc + 1])
                    if tb == 0:
                        # stream in the remaining resident W1 blocks 2-ahead;
                        # xb(tb1) is not needed until the tb1 sweep
                        if fb == 0:
                            nc.sync.dma_start(out=w1t[1], in_=w1sp[1])
                            nc.sync.dma_start(out=w1t[2], in_=w1sp[2])
                        elif fb < 6:
                            nc.sync.dma_start(out=w1t[fb + 2], in_=w1sp[fb + 2])
                        if fb == 2:
                            nc.sync.dma_start(out=xb[:, 1], in_=xbp[1])

            for _ in el1:  # drain any remaining expert L1 ops (should be ~0)
                pass

            # ---- layer 2 -------------------------------------------------
            # shared path, transposed output: 128-wide d slices, streamed
            # bf16 W2 chunks; fp8 DR expert units paced between shared MMs.
            def fill_w2ch(dc):
                w2ch = pool.tile([P, KO2, 128], bf16, tag="w2ch", bufs=3,
                                 name=f"w2ch_{dc}")
                nc.sync.dma_start(out=w2ch, in_=w2sp[dc])
                return w2ch

            units = [(le, dc) for le in range(E_PER_CORE) for dc in range(KO1)]

            def load_w2e8(le, dc):
                t = pool.tile([P, KO2, 128], fp8, tag="w2e8", bufs=3,
                              name=f"w2e8_{le}_{dc}")
                nc.gpsimd.dma_start(out=t, in_=w2ep8[le, dc])
                return t

            obeT = pool.tile([P, KO1, E_PER_CORE * BATCH], bf16, tag="obeT",
                             bufs=1)

            def expert_l2_gen():
                nxt = load_w2e8(*units[0])
                yield  # priming
                for idx, (le, dc) in enumerate(units):
                    w2e = nxt
                    nxt = (load_w2e8(*units[idx + 1])
                           if idx + 1 < len(units) else None)
                    pse2 = pp.tile([P, BATCH], f32, tag="pse2", bufs=2,
                                   name=f"pse2_{le}_{dc}")
                    for k2 in range(0, KO2, 2):
                        nc.tensor.matmul(
                            pse2,
                            w2e[:, k2:k2 + 2, :],
                            heT8[le][:, k2:k2 + 2, :],
                            start=(k2 == 0), stop=(k2 == KO2 - 2),
                            perf_mode=DR)
                        yield
                    nc.scalar.activation(
                        obeT[:, dc, le * BATCH:(le + 1) * BATCH], pse2,
                        AF.Identity, bias=b2e_sb[le][:, dc:dc + 1],
                        scale=1.0 / W2S)
                # all 16 units done: ns write overlaps chunk 7
                nc.gpsimd.dma_start(out=outnsT[:], in_=obeT)

            el2 = expert_l2_gen()
            next(el2, None)  # prime unit-0 weights load during L1 tail

            chs = {0: fill_w2ch(0), 1: fill_w2ch(1), 2: fill_w2ch(2)}
            for sc in range(8):
                for tb in range(TBLK):
                    ps2 = pp.tile([P, 512], f32, tag="ps2", bufs=2,
                                  name=f"ps2_{sc}_{tb}")
                    for k in range(KO2):
                        nc.tensor.matmul(
                            ps2,
                            chs[sc][:, k, :],
                            hT[:, k, tb * 512:(tb + 1) * 512],
                            start=(k == 0), stop=(k == KO2 - 1))
                        if k % 2 == 1 or sc in (5, 6):
                            next(el2, None)
                    ot = pool.tile([P, 512], bf16, tag="ot", bufs=2,
                                   name=f"ot_{sc}_{tb}")
                    nc.scalar.activation(ot, ps2, AF.Identity,
                                         bias=b2s_sb[:, sc:sc + 1])
                    nc.sync.dma_start(
                        out=outsT[sc * 128:(sc + 1) * 128,
                                  tb * 512:(tb + 1) * 512],
                        in_=ot)
                if sc == 6:
                    for _ in el2:  # drain stragglers + emit the ns write
                        pass
                if sc + 3 < 8:
                    chs[sc + 3] = fill_w2ch(sc + 3)

    nc.compile()
    return nc


def _get_nc():
    if "nc" not in _state:
        _state["nc"] = _build()
    return _state["nc"]


def _f32(a):
    return np.ascontiguousarray(np.asarray(a, dtype=np.float32))


def kernel(x, W1_seq, b1_seq, W2_seq, b2_seq, W1_ns, b1_ns, W2_ns, b2_ns,
           seq_token_count):
    from concourse.bass_utils import run_bass_kernel_spmd

    assert int(seq_token_count) == SEQ_TOK
    xf = np.asarray(x, dtype=np.float32)
    xb16 = xf.astype(BF16)
    W1sb = np.asarray(W1_seq, dtype=np.float32).astype(BF16)
    W2sb = np.asarray(W2_seq, dtype=np.float32).astype(BF16)
    W1n8 = (np.asarray(W1_ns, dtype=np.float32) * W1S).astype(E4)
    W2n8 = (np.asarray(W2_ns, dtype=np.float32) * W2S).astype(E4)
    b1_seq, b2_seq = _f32(b1_seq), _f32(b2_seq)
    b1_ns, b2_ns = _f32(b1_ns), _f32(b2_ns)

    nc = _get_nc()

    # host-side (lossless) re-layouts: contraction dim on partitions, then
    # piece-major packing so each device DMA is one contiguous read
    # w1sp[fb, p, fs, kc, fj] = W1_seq[kc*128+p, fb*512+fs*128+fj]
    w1sp_h = np.ascontiguousarray(
        W1sb.reshape(KO1, P, FBLK, 4, 128).transpose(2, 1, 3, 0, 4))
    # w2sp[dc, p, kc, di] = W2_seq[kc*128+p, dc*128+di]
    w2sp_h = np.ascontiguousarray(
        W2sb.reshape(KO2, P, 8, 128).transpose(2, 1, 0, 3))
    b1s_h = np.ascontiguousarray(b1_seq.reshape(KO2, P).T)          # [P, KO2]
    b2s_h = np.ascontiguousarray(b2_seq.reshape(KO1, P).T)          # [P, KO1]

    in_maps = []
    for i in range(N_CORES):
        # xbp[tb, p, kc, ti] = x[i, tb*512+ti, kc*128+p]
        xT = xb16[i, :SEQ_TOK, :].T                                 # [D, T]
        xbp_h = np.ascontiguousarray(
            xT.reshape(KO1, P, TBLK, 512).transpose(2, 1, 0, 3))
        # xnsT8[p, kc, le*8+b] = 16 * x[b, 1024 + 2i + le, kc*128+p]
        xns = (xf[:, SEQ_TOK + 2 * i:SEQ_TOK + 2 * i + 2, :] * XS).astype(E4)
        xnsT8_h = np.ascontiguousarray(
            xns.transpose(2, 1, 0).reshape(KO1, P, E_PER_CORE, BATCH)
            .transpose(1, 0, 2, 3).reshape(P, KO1, E_PER_CORE * BATCH))
        # w1ep8[le, fb, p, fs, kc, fj] = 32*W1_ns[2i+le, kc*128+p, fb*512+fs*128+fj]
        w1ep8_h = np.ascontiguousarray(
            W1n8[2 * i:2 * i + 2].reshape(E_PER_CORE, KO1, P, FBLK, 4, 128)
            .transpose(0, 3, 2, 4, 1, 5))
        # w2ep8[le, dc, p, kc, di] = 64*W2_ns[2i+le, kc*128+p, dc*128+di]
        w2ep8_h = np.ascontiguousarray(
            W2n8[2 * i:2 * i + 2].reshape(E_PER_CORE, KO2, P, KO1, 128)
            .transpose(0, 3, 2, 1, 4))
        b1e_h = b1_ns[2 * i:2 * i + 2].reshape(E_PER_CORE, KO2, P)
        b2e_h = b2_ns[2 * i:2 * i + 2].reshape(E_PER_CORE, KO1, P)
        consts_h = np.ascontiguousarray(np.concatenate([
            b1s_h, b2s_h, b1e_h[0].T, b1e_h[1].T, b2e_h[0].T, b2e_h[1].T,
        ], axis=1))
        in_maps.append({
            "xbp": xbp_h, "xnsT8": xnsT8_h,
            "w1sp": w1sp_h, "w2sp": w2sp_h, "consts": consts_h,
            "w1ep8": w1ep8_h, "w2ep8": w2ep8_h,
        })

    trace = bool(int(os.environ.get("KERNEL_TRACE", "0")))
    kw = {}
    if trace:
        kw["trace"] = True
        tc_env = os.environ.get("KERNEL_TRACE_CORES", "0")
        kw["trace_cores"] = [int(c) for c in tc_env.split(",")]
    res = run_bass_kernel_spmd(nc, in_maps, list(range(N_CORES)), **kw)
    _state["last_result"] = res

    out = np.empty((BATCH, SEQ_LEN, D_MODEL), np.float32)
    for i in range(N_CORES):
        out[i, :SEQ_TOK, :] = res.results[i]["outsT"].astype(np.float32).T
        # outnsT[p, dc, le*8+b] = out[b, 1024+2i+le, dc*128+p]
        ns = (res.results[i]["outnsT"].astype(np.float32)
              .transpose(2, 1, 0).reshape(E_PER_CORE, BATCH, D_MODEL))
        out[:, SEQ_TOK + 2 * i, :] = ns[0]
        out[:, SEQ_TOK + 2 * i + 1, :] = ns[1]
    return out


# revision 16
# speedup vs baseline: 1.0136x; 1.0136x over previous
"""Trainium2 Bass kernel for nn_MixedFeedForward (shared MLP + 16 per-ns-token MLPs).

Sharding (8 NeuronCores, SPMD, no collectives):
  - shared path: data-parallel over batch -> core i runs the shared MLP over
    x[i, :1024, :].
  - ns path: expert-parallel -> core i runs experts {2i, 2i+1}, each over the
    8 batches' single ns token for that expert.
Each core writes a disjoint slice of the output; the host assembles.

Shared path is bf16 (PE streaming floor ~221us/core); expert path is
fp8e4 + DoubleRow (halves the 1024 expert matmuls; ns tokens are 16/1040
of the output so the fp8 noise contributes ~7e-3 global rel err).

Schedule (all paced by emission order, PE FIFO):
  - ~8 dummy N=512 matmuls at t~7us warm the HAM clock gate before real
    work; short dummy fillers in the first f-blocks keep it warm through
    the DMA-starved start.
  - L1 runs token-block 0 across all 8 f-blocks, then token-block 1
    (W1_seq held resident in SBUF) so the early DMA demand is just
    256KB/1.7us after the first group.
  - Expert L1/L2 DoubleRow matmuls are Bresenham-paced between shared
    matmuls (~1 per 1.75) so their 256-col LDWEIGHTS hide under the
    216ns shared streams; expert output is written during chunk 7.
"""

import os
import sys
import numpy as np
import ml_dtypes

BF16 = ml_dtypes.bfloat16
E4 = ml_dtypes.float8_e4m3

P = 128
D_MODEL, D_FF = 1024, 4096
SEQ_TOK, NS_TOK, BATCH = 1024, 16, 8
SEQ_LEN = SEQ_TOK + NS_TOK
N_CORES = 8
E_PER_CORE = 2
KO1 = D_MODEL // P      # 8  k-chunks when contracting over d_model
KO2 = D_FF // P         # 32 k-chunks when contracting over d_ff
FBLK = D_FF // 512      # 8  f-blocks (512 wide)
TBLK = SEQ_TOK // 512   # 2  token blocks (512 wide)
XS, W1S, W2S = 16.0, 32.0, 64.0   # fp8 pre-scales (powers of 2)

_state = {}


def _ensure_axon_profile_hook():
    """Some agent images lack antenv.axon_hooks; provide a shim so
    run_bass_kernel_spmd(trace=True) can capture NTFF profiles via the
    libaxon_pjrt C ABI (same mechanism as trn_agent_boot)."""
    try:
        import antenv.axon_hooks  # noqa: F401
        return
    except ImportError:
        pass
    import contextlib
    import ctypes
    import types

    so_path = "/opt/axon/libaxon_pjrt.so"
    hook = None
    if os.path.exists(so_path):
        try:
            lib = ctypes.CDLL(so_path)
            if hasattr(lib, "axon_start_nrt_profile"):
                lib.axon_start_nrt_profile.argtypes = [
                    ctypes.POINTER(ctypes.c_int64), ctypes.c_size_t]
                lib.axon_start_nrt_profile.restype = ctypes.c_int64
                lib.axon_stop_nrt_profile.argtypes = [ctypes.c_char_p]
                lib.axon_stop_nrt_profile.restype = ctypes.c_int64

                @contextlib.contextmanager
                def _hook(output_dir, device_ids):
                    import jax
                    jax.devices()
                    if device_ids:
                        ids = (ctypes.c_int64 * len(device_ids))(*device_ids)
                        rc = lib.axon_start_nrt_profile(ids, len(device_ids))
                    else:
                        rc = lib.axon_start_nrt_profile(None, 0)
                    if rc != 0:
                        raise RuntimeError(f"axon_start_nrt_profile rc={rc}")
                    try:
                        yield
                    finally:
                        n = lib.axon_stop_nrt_profile(str(output_dir).encode())
                        print(f"profile: {n} file(s) written to {output_dir}",
                              file=sys.stderr)

                hook = _hook
        except OSError:
            pass

    mod = types.ModuleType("antenv.axon_hooks")
    _store = {"hook": hook}
    mod.set_axon_ntff_profile_hook = lambda h: _store.__setitem__("hook", h)
    mod.get_axon_ntff_profile_hook = lambda: _store["hook"]
    sys.modules["antenv.axon_hooks"] = mod


_ensure_axon_profile_hook()


def _build():
    import concourse.mybir as mybir
    import concourse.tile as tile
    from concourse import bacc

    f32 = mybir.dt.float32
    bf16 = mybir.dt.bfloat16
    fp8 = mybir.dt.float8e4
    AF = mybir.ActivationFunctionType
    DR = mybir.MatmulPerfMode.DoubleRow

    nc = bacc.Bacc(None, target_bir_lowering=False, debug=False)

    # piece-major DRAM layouts: every weight/x DMA below is one fully
    # contiguous read
    xbp = nc.dram_tensor("xbp", [TBLK, P, KO1, 512], bf16, kind="ExternalInput")
    xnsT8 = nc.dram_tensor("xnsT8", [P, KO1, E_PER_CORE * BATCH], fp8,
                           kind="ExternalInput")
    # W1 pieces are fs-major so the warm-up can stream 256KB sub-pieces
    w1sp = nc.dram_tensor("w1sp", [FBLK, P, 4, KO1, 128], bf16, kind="ExternalInput")
    w2sp = nc.dram_tensor("w2sp", [8, P, KO2, 128], bf16, kind="ExternalInput")
    w1ep8 = nc.dram_tensor("w1ep8", [E_PER_CORE, FBLK, P, 4, KO1, 128], fp8,
                           kind="ExternalInput")
    w2ep8 = nc.dram_tensor("w2ep8", [E_PER_CORE, KO1, P, KO2, 128], fp8,
                           kind="ExternalInput")
    # all per-partition bias constants packed into one contiguous DMA:
    # cols [0:32)=b1s [32:40)=b2s [40:72)=b1e0 [72:104)=b1e1
    #      [104:112)=b2e0 [112:120)=b2e1
    consts = nc.dram_tensor("consts", [P, 120], f32, kind="ExternalInput")
    outsT = nc.dram_tensor("outsT", [D_MODEL, SEQ_TOK], bf16, kind="ExternalOutput")
    outnsT = nc.dram_tensor("outnsT", [P, KO1, E_PER_CORE * BATCH], bf16,
                            kind="ExternalOutput")

    with tile.TileContext(nc) as tc:
        with tc.tile_pool(name="main", bufs=1) as pool, \
             tc.tile_pool(name="psum", bufs=1, space="PSUM") as pp:

            # ---- HAM pre-warm: dummy matmuls with no DMA deps keep the PE
            # busy from ~8us so the clock gate is at 8/8 when real MMs start.
            # psd shares the pse2 bank (idle until L2). gpsimd memset runs
            # ~0.7us earlier than vector (shorter preamble).
            dummy = pool.tile([P, 512], bf16, tag="dummy", bufs=1)
            nc.gpsimd.memset(dummy, 0.0)
            psd = pp.tile([P, 512], f32, tag="pse2", bufs=2, name="psd")
            for i in range(10):
                nc.tensor.matmul(psd, dummy[:, 0:128], dummy,
                                 start=True, stop=True)

            def dummy_mm():
                # short (N=128) filler to keep the PE busy across early
                # DMA-starved stalls without delaying real MMs much
                nc.tensor.matmul(psd[:, 0:128], dummy[:, 0:128],
                                 dummy[:, 0:128], start=True, stop=True)

            # ---- persistent activations / resident weights ---------------
            xb = pool.tile([P, TBLK, KO1, 512], bf16, tag="xb", bufs=1)
            hT = pool.tile([P, KO2, SEQ_TOK], bf16, tag="hT", bufs=1)
            heT8 = []
            for le in range(E_PER_CORE):
                t = pool.tile([P, KO2, BATCH], fp8, tag=f"heT{le}", bufs=1,
                              name=f"heT{le}")
                heT8.append(t)
            w1t = [pool.tile([P, 4, KO1, 128], bf16, tag="w1res", bufs=FBLK,
                             name=f"w1res{fb}") for fb in range(FBLK)]

            # ---- warm-up DMAs, dispatched on 3 engines in parallel -------
            # (each dma_start costs ~0.7us of serial DIRECT2D dispatch on
            # its engine; sync/scalar/gpsimd chains run concurrently)
            nc.sync.dma_start(out=xb[:, 0, 0:2, :], in_=xbp[0][:, 0:2])
            nc.scalar.dma_start(out=w1t[0][:, 0], in_=w1sp[0][:, 0])
            nc.sync.dma_start(out=xb[:, 0, 2:4, :], in_=xbp[0][:, 2:4])
            nc.scalar.dma_start(out=w1t[0][:, 1], in_=w1sp[0][:, 1])
            nc.sync.dma_start(out=xb[:, 0, 4:6, :], in_=xbp[0][:, 4:6])
            nc.scalar.dma_start(out=w1t[0][:, 2], in_=w1sp[0][:, 2])
            nc.sync.dma_start(out=xb[:, 0, 6:8, :], in_=xbp[0][:, 6:8])
            nc.scalar.dma_start(out=w1t[0][:, 3], in_=w1sp[0][:, 3])
            cs = pool.tile([P, 120], f32, tag="consts", bufs=1)
            nc.gpsimd.dma_start(out=cs, in_=consts[:])
            b1s_sb = cs[:, 0:32]
            b2s_sb = cs[:, 32:40]
            b1e_sb = [cs[:, 40:72], cs[:, 72:104]]
            b2e_sb = [cs[:, 104:112], cs[:, 112:120]]
            xnsb8 = pool.tile([P, KO1, E_PER_CORE * BATCH], fp8, tag="xnsb8",
                              bufs=1)
            nc.gpsimd.dma_start(out=xnsb8, in_=xnsT8[:])

            # ---- expert L1: fp8 DoubleRow stream (one yield per MM) ------
            l1_rounds = [(le, rfb) for le in range(E_PER_CORE)
                         for rfb in range(FBLK)]

            def load_we8(le, rfb):
                t = pool.tile([P, 4, KO1, 128], fp8, tag="wb8", bufs=3,
                              name=f"we8_{le}_{rfb}")
                nc.scalar.dma_start(out=t, in_=w1ep8[le, rfb])
                return t

            def expert_l1_gen():
                nxt = load_we8(*l1_rounds[0])
                yield  # priming: weights for round 0 issued, no MM yet
                for idx, (le, rfb) in enumerate(l1_rounds):
                    web = nxt
                    nxt = (load_we8(*l1_rounds[idx + 1])
                           if idx + 1 < len(l1_rounds) else None)
                    for fs in range(4):
                        fc = rfb * 4 + fs
                        pse = pp.tile([P, BATCH], f32, tag="pse1", bufs=2,
                                      name=f"pse1_{le}_{fc}")
                        for k2 in range(0, KO1, 2):
                            nc.tensor.matmul(
                                pse,
                                web[:, fs, k2:k2 + 2, :],
                                xnsb8[:, k2:k2 + 2,
                                      le * BATCH:(le + 1) * BATCH],
                                start=(k2 == 0), stop=(k2 == KO1 - 2),
                                perf_mode=DR)
                            yield
                        nc.scalar.activation(
                            heT8[le][:, fc, :], pse, AF.Gelu,
                            bias=b1e_sb[le][:, fc:fc + 1],
                            scale=1.0 / (XS * W1S))

            el1 = expert_l1_gen()

            # ---- layer 1: tb0 sweep then tb1 sweep (W1 resident) ---------
            # Expert DR tiles strictly every 2nd shared MM (a 256-col DR
            # LDWEIGHTS only hides under a full 216ns shared MM stream);
            # extra stride-1 slots late in pass2 absorb the remainder.
            for tb in range(TBLK):
                for fb in range(FBLK):
                    for fs in range(4):
                        fc = fb * 4 + fs
                        ps1 = pp.tile([P, 512], f32, tag="ps1", bufs=2,
                                      name=f"ps1_{fc}_{tb}")
                        for k in range(KO1):
                            nc.tensor.matmul(
                                ps1,
                                w1t[fb][:, fs, k, :],
                                xb[:, tb, k, :],
                                start=(k == 0), stop=(k == KO1 - 1))
                            # dense dummy fillers through the DMA-starved
                            # start keep the HAM clock warm (they only cost
                            # time if the data was on time, which it isn't)
                            if tb == 0 and fb == 0:
                                dummy_mm()
                            elif tb == 0 and fb == 1:
                                if k % 2 == 1:
                                    dummy_mm()
                                    next(el1, None)
                            elif tb == 0 and fb == 2:
                                if k % 4 == 3:
                                    dummy_mm()
                                if k % 2 == 1:
                                    next(el1, None)
                            elif k % 2 == 1 or (tb == 1 and fb >= 6):
                                next(el1, None)
                        nc.scalar.activation(
                            hT[:, fc, tb * 512:(tb + 1) * 512], ps1, AF.Gelu,
                            bias=b1s_sb[:, fc:fc + 1])
                        if tb == 0 and fb == 0 and fs == 1:
                            nc.sync.dma_start(out=w1t[1][:, 0:2], in_=w1sp[1][:, 0:2])
                            nc.sync.dma_start(out=w1t[1][:, 2:4], in_=w1sp[1][:, 2:4])
                        if tb == 0 and fb == 0 and fs == 3:
                            nc.sync.dma_start(out=w1t[2][:, 0:2], in_=w1sp[2][:, 0:2])
                            nc.sync.dma_start(out=w1t[2][:, 2:4], in_=w1sp[2][:, 2:4])
                            next(el1, None)  # prime round-0 expert load
                    if tb == 0:
                        # stream in the remaining resident W1 blocks; defer
                        # anything not needed soon out of the starved window
                        if 0 < fb < 6:
                            nc.sync.dma_start(out=w1t[fb + 2], in_=w1sp[fb + 2])
                        if fb == 2:
                            nc.sync.dma_start(out=xb[:, 1], in_=xbp[1])

            for _ in el1:  # drain any remaining expert L1 ops (should be ~0)
                pass

            # ---- layer 2 -------------------------------------------------
            # shared path, transposed output: 128-wide d slices, streamed
            # bf16 W2 chunks; fp8 DR expert units paced between shared MMs.
            def fill_w2ch(dc):
                w2ch = pool.tile([P, KO2, 128], bf16, tag="w2ch", bufs=3,
                                 name=f"w2ch_{dc}")
                nc.sync.dma_start(out=w2ch, in_=w2sp[dc])
                return w2ch

            units = [(le, dc) for le in range(E_PER_CORE) for dc in range(KO1)]

            def load_w2e8(le, dc):
                t = pool.tile([P, KO2, 128], fp8, tag="w2e8", bufs=3,
                              name=f"w2e8_{le}_{dc}")
                nc.scalar.dma_start(out=t, in_=w2ep8[le, dc])
                return t

            obeT = pool.tile([P, KO1, E_PER_CORE * BATCH], bf16, tag="obeT",
                             bufs=1)

            def expert_l2_gen():
                nxt = load_w2e8(*units[0])
                yield  # priming
                for idx, (le, dc) in enumerate(units):
                    w2e = nxt
                    nxt = (load_w2e8(*units[idx + 1])
                           if idx + 1 < len(units) else None)
                    pse2 = pp.tile([P, BATCH], f32, tag="pse2", bufs=2,
                                   name=f"pse2_{le}_{dc}")
                    for k2 in range(0, KO2, 2):
                        nc.tensor.matmul(
                            pse2,
                            w2e[:, k2:k2 + 2, :],
                            heT8[le][:, k2:k2 + 2, :],
                            start=(k2 == 0), stop=(k2 == KO2 - 2),
                            perf_mode=DR)
                        yield
                    nc.scalar.activation(
                        obeT[:, dc, le * BATCH:(le + 1) * BATCH], pse2,
                        AF.Identity, bias=b2e_sb[le][:, dc:dc + 1],
                        scale=1.0 / W2S)
                # all 16 units done: ns write overlaps chunk 7
                nc.gpsimd.dma_start(out=outnsT[:], in_=obeT)

            el2 = expert_l2_gen()
            next(el2, None)  # prime unit-0 weights load during L1 tail

            chs = {0: fill_w2ch(0), 1: fill_w2ch(1), 2: fill_w2ch(2)}
            for sc in range(8):
                for tb in range(TBLK):
                    ps2 = pp.tile([P, 512], f32, tag="ps2", bufs=2,
                                  name=f"ps2_{sc}_{tb}")
                    for k in range(KO2):
                        nc.tensor.matmul(
                            ps2,
                            chs[sc][:, k, :],
                            hT[:, k, tb * 512:(tb + 1) * 512],
                            start=(k == 0), stop=(k == KO2 - 1))
                        if k % 2 == 1 or sc in (5, 6):
                            next(el2, None)
                    ot = pool.tile([P, 512], bf16, tag="ot", bufs=2,
                                   name=f"ot_{sc}_{tb}")
                    nc.scalar.activation(ot, ps2, AF.Identity,
                                         bias=b2s_sb[:, sc:sc + 1])
                    nc.sync.dma_start(
                        out=outsT[sc * 128:(sc + 1) * 128,
                                  tb * 512:(tb + 1) * 512],
                        in_=ot)
                if sc == 6:
                    for _ in el2:  # drain stragglers + emit the ns write
                        pass
                if sc + 3 < 8:
                    chs[sc + 3] = fill_w2ch(sc + 3)

    nc.compile()
    return nc


def _get_nc():
    if "nc" not in _state:
        _state["nc"] = _build()
    return _state["nc"]


def _f32(a):
    return np.ascontiguousarray(np.asarray(a, dtype=np.float32))


def kernel(x, W1_seq, b1_seq, W2_seq, b2_seq, W1_ns, b1_ns, W2_ns, b2_ns,
           seq_token_count):
    from concourse.bass_utils import run_bass_kernel_spmd

    assert int(seq_token_count) == SEQ_TOK
    xf = np.asarray(x, dtype=np.float32)
    xb16 = xf.astype(BF16)
    W1sb = np.asarray(W1_seq, dtype=np.float32).astype(BF16)
    W2sb = np.asarray(W2_seq, dtype=np.float32).astype(BF16)
    W1n8 = (np.asarray(W1_ns, dtype=np.float32) * W1S).astype(E4)
    W2n8 = (np.asarray(W2_ns, dtype=np.float32) * W2S).astype(E4)
    b1_seq, b2_seq = _f32(b1_seq), _f32(b2_seq)
    b1_ns, b2_ns = _f32(b1_ns), _f32(b2_ns)

    nc = _get_nc()

    # host-side (lossless) re-layouts: contraction dim on partitions, then
    # piece-major packing so each device DMA is one contiguous read
    # w1sp[fb, p, fs, kc, fj] = W1_seq[kc*128+p, fb*512+fs*128+fj]
    w1sp_h = np.ascontiguousarray(
        W1sb.reshape(KO1, P, FBLK, 4, 128).transpose(2, 1, 3, 0, 4))
    # w2sp[dc, p, kc, di] = W2_seq[kc*128+p, dc*128+di]
    w2sp_h = np.ascontiguousarray(
        W2sb.reshape(KO2, P, 8, 128).transpose(2, 1, 0, 3))
    b1s_h = np.ascontiguousarray(b1_seq.reshape(KO2, P).T)          # [P, KO2]
    b2s_h = np.ascontiguousarray(b2_seq.reshape(KO1, P).T)          # [P, KO1]

    in_maps = []
    for i in range(N_CORES):
        # xbp[tb, p, kc, ti] = x[i, tb*512+ti, kc*128+p]
        xT = xb16[i, :SEQ_TOK, :].T                                 # [D, T]
        xbp_h = np.ascontiguousarray(
            xT.reshape(KO1, P, TBLK, 512).transpose(2, 1, 0, 3))
        # xnsT8[p, kc, le*8+b] = 16 * x[b, 1024 + 2i + le, kc*128+p]
        xns = (xf[:, SEQ_TOK + 2 * i:SEQ_TOK + 2 * i + 2, :] * XS).astype(E4)
        xnsT8_h = np.ascontiguousarray(
            xns.transpose(2, 1, 0).reshape(KO1, P, E_PER_CORE, BATCH)
            .transpose(1, 0, 2, 3).reshape(P, KO1, E_PER_CORE * BATCH))
        # w1ep8[le, fb, p, fs, kc, fj] = 32*W1_ns[2i+le, kc*128+p, fb*512+fs*128+fj]
        w1ep8_h = np.ascontiguousarray(
            W1n8[2 * i:2 * i + 2].reshape(E_PER_CORE, KO1, P, FBLK, 4, 128)
            .transpose(0, 3, 2, 4, 1, 5))
        # w2ep8[le, dc, p, kc, di] = 64*W2_ns[2i+le, kc*128+p, dc*128+di]
        w2ep8_h = np.ascontiguousarray(
            W2n8[2 * i:2 * i + 2].reshape(E_PER_CORE, KO2, P, KO1, 128)
            .transpose(0, 3, 2, 1, 4))
        b1e_h = b1_ns[2 * i:2 * i + 2].reshape(E_PER_CORE, KO2, P)
        b2e_h = b2_ns[2 * i:2 * i + 2].reshape(E_PER_CORE, KO1, P)
        consts_h = np.ascontiguousarray(np.concatenate([
            b1s_h, b2s_h, b1e_h[0].T, b1e_h[1].T, b2e_h[0].T, b2e_h[1].T,
        ], axis=1))
        in_maps.append({
            "xbp": xbp_h, "xnsT8": xnsT8_h,
            "w1sp": w1sp_h, "w2sp": w2sp_h, "consts": consts_h,
            "w1ep8": w1ep8_h, "w2ep8": w2ep8_h,
        })

    trace = bool(int(os.environ.get("KERNEL_TRACE", "0")))
    kw = {}
    if trace:
        kw["trace"] = True
        tc_env = os.environ.get("KERNEL_TRACE_CORES", "0")
        kw["trace_cores"] = [int(c) for c in tc_env.split(",")]
    res = run_bass_kernel_spmd(nc, in_maps, list(range(N_CORES)), **kw)
    _state["last_result"] = res

    out = np.empty((BATCH, SEQ_LEN, D_MODEL), np.float32)
    for i in range(N_CORES):
        out[i, :SEQ_TOK, :] = res.results[i]["outsT"].astype(np.float32).T
        # outnsT[p, dc, le*8+b] = out[b, 1024+2i+le, dc*128+p]
        ns = (res.results[i]["outnsT"].astype(np.float32)
              .transpose(2, 1, 0).reshape(E_PER_CORE, BATCH, D_MODEL))
        out[:, SEQ_TOK + 2 * i, :] = ns[0]
        out[:, SEQ_TOK + 2 * i + 1, :] = ns[1]
    return out
